# revision 24
# baseline (speedup 1.0000x reference)
"""Trainium2 Bass kernel for nn_BoundaryHead (nms_detection).

Data-parallel over batch B=64 across 8 NeuronCores (8 rows/core). Each core:
  P1: three linear heads via PE (fp32): x [8,2048,256] is PE-transposed in
      128x128 blocks to put D on partitions, then matmul with packed W [256,3]
      accumulating into PSUM [3, 2048] per row; PSUM->SBUF via ACT, then DMA
      rows into [8, T] per-head tensors.
  P2: sigmoid+bias+saliency mask.
  P3: NMS maxpool(3) via two shifted maxes; cp = (hmax==center)*center.
  P4: Gaussian targets: per-object 40-wide profiles + indirect-DMA scatter-max
      into DRAM; window/offset/weight value scatters (last-write-wins, OOB
      drop for invalid); DMA back.
  P5: GaussianFocal + L1 loss partial sums per row (division by the global
      avg_factor happens on host after the cross-core gather).
  P6: topk-100 via pairwise-reduced [8,1024] array, 13 rounds of
      max8/max_index/match_replace (exact, incl. duplicate handling).
  P7: dense lo/hi decode, pair payload -> DRAM, indirect gather at top
      indices, boundary assembly.

Host only shards inputs, sums the per-row loss partials, and divides by the
all-reduced avg_factor (the one cross-core reduction in this model).
"""

import numpy as np

import concourse.bass as bass
import concourse.mybir as mybir
import concourse.tile as tile
from concourse import bacc
from concourse.bass import IndirectOffsetOnAxis
from concourse.bass_utils import run_bass_kernel_spmd
from concourse.masks import make_identity

F32 = mybir.dt.float32
U32 = mybir.dt.uint32
I32 = mybir.dt.int32
A = mybir.AluOpType
AF = mybir.ActivationFunctionType
AX = mybir.AxisListType

B, T, D, M = 64, 2048, 256, 20
NCORES = 8
R = B // NCORES          # 8 batch rows per core
TOPK = 100
K104 = 104               # 13 rounds x 8
NR = 13
W = 40                   # gaussian profile width (|d| <= 19)
PAD = 32                 # per-row padding in ct scratch
TP = T + 2 * PAD         # 2112
EPS = 1e-12
MAGIC = float(2 ** 23)
NEG = -1e30


def _floor(nc, sb, x, name):
    """Exact floor for 0 <= x < 2^23 via round-to-nearest magic + fixup."""
    r = sb.tile([80, 1], F32, name=f"{name}_r", tag=f"{name}_r")
    g = sb.tile([80, 1], F32, name=f"{name}_g", tag=f"{name}_g")
    nc.vector.tensor_scalar(r[:], x, MAGIC, None, op0=A.add)
    nc.vector.tensor_scalar(r[:], r[:], MAGIC, None, op0=A.subtract)
    nc.vector.tensor_tensor(g[:], r[:], x, op=A.is_gt)
    nc.vector.tensor_tensor(r[:], r[:], g[:], op=A.subtract)
    return r


def build_nc():
    nc = bacc.Bacc("TRN2", target_bir_lowering=False, debug=False,
                   num_devices=NCORES)

    x_d = nc.dram_tensor("x", [R, T, D], F32, kind="ExternalInput")
    sal_d = nc.dram_tensor("sal", [R, T], F32, kind="ExternalInput")
    bnd_d = nc.dram_tensor("bnd", [R, M, 2], F32, kind="ExternalInput")
    wpk_d = nc.dram_tensor("wpk", [D, 3], F32, kind="ExternalInput")
    brep_d = nc.dram_tensor("brep", [R, 3], F32, kind="ExternalInput")

    obnd_d = nc.dram_tensor("obnd", [R, TOPK, 3], F32, kind="ExternalOutput")
    opart_d = nc.dram_tensor("opart", [R, 4], F32, kind="ExternalOutput")

    with tile.TileContext(nc) as tc:
        with (
            tc.tile_pool(name="cst", bufs=1) as cst,
            tc.tile_pool(name="sb", bufs=1) as sb,
            tc.tile_pool(name="tmp", bufs=1) as tmp,
            tc.tile_pool(name="xin", bufs=2) as xin,
            tc.tile_pool(name="xts", bufs=2) as xts,
            tc.tile_pool(name="psA", bufs=2, space="PSUM") as psA,
            tc.tile_pool(name="psB", bufs=1, space="PSUM") as psB,
            tc.tile_pool(name="psC", bufs=1, space="PSUM") as psC,
            tc.tile_pool(name="dram", bufs=1, space="DRAM") as dram,
        ):
            # ---------------- DRAM scratch (dep-tracked pool tiles) --------
            wtg_d = dram.tile([R * T], F32)
            otg_d = dram.tile([R * T], F32)
            wgt_d = dram.tile([R * T], F32)
            pay_d = dram.tile([R * T * 3], F32)      # pair payload, 6/pair

            # ---------------- constants -----------------------------------
            ident = cst.tile([128, 128], F32)
            make_identity(nc, ident[:])

            w_sb = cst.tile([128, 6], F32)
            nc.sync.dma_start(w_sb[:, 0:3], wpk_d[0:128, :])
            nc.sync.dma_start(w_sb[:, 3:6], wpk_d[128:256, :])
            brep_sb = cst.tile([R, 3], F32)
            nc.sync.dma_start(brep_sb[:], brep_d[:])
            sal_sb = sb.tile([R, T], F32, name="sal_sb", tag="big")
            nc.sync.dma_start(sal_sb[:], sal_d[:])

            # p -> b = p % 8 for the [80] (m-major) group layout
            pio = cst.tile([80, 1], I32)
            nc.gpsimd.iota(pio[:], [[1, 1]], base=0, channel_multiplier=1)
            piof = cst.tile([80, 1], F32)
            nc.vector.tensor_copy(piof[:], pio[:])
            p8 = cst.tile([80, 1], F32)
            nc.vector.tensor_scalar(p8[:], piof[:], 0.1, None, op0=A.mult)
            bidx = _floor(nc, cst, p8[:], "p8f")   # b of each (b, m) row
            rb2048 = cst.tile([80, 1], F32)
            nc.vector.tensor_scalar(rb2048[:], bidx[:], float(T), None, op0=A.mult)

            # gaussian broadcast helpers
            ones1 = cst.tile([1, 128], F32)
            nc.vector.memset(ones1[:], 1.0)
            tpcol = cst.tile([128, 1], F32)
            nc.gpsimd.iota(tpcol[:], [[1, 1]], base=0, channel_multiplier=1,
                           allow_small_or_imprecise_dtypes=True)

            # activation bias constants (must be APs)
            epsb = cst.tile([R, 1], F32)
            nc.vector.memset(epsb[:], EPS)
            lneps = cst.tile([R, 1], F32)
            nc.vector.memset(lneps[:], 1.0 + EPS)
            twob = cst.tile([R, 1], F32)
            nc.vector.memset(twob[:], 2.0)

            # iotas for decode
            rb1024 = cst.tile([R, 1], F32)
            nc.gpsimd.iota(rb1024[:], [[1, 1]], base=0, channel_multiplier=1024,
                           allow_small_or_imprecise_dtypes=True)

            # ---------------- zero-init DRAM scratch -----------------------
            zer = tmp.tile([R, TP], F32, name="zer", tag="E")
            nc.gpsimd.memset(zer[:], 0.0)
            for zd in (wtg_d, otg_d, wgt_d):
                nc.sync.dma_start(zd[:].rearrange("(r t) -> r t", t=T),
                                  zer[:, 0:T])

            # ---------------- P1: heads -----------------------------------
            clog = sb.tile([R, T], F32)
            wlog = sb.tile([R, T], F32)
            olog = sb.tile([R, T], F32)
            heads = (clog, wlog, olog)
            for bi in range(R):
                pp_sb = sb.tile([3, T], F32, name="pp_sb", tag="sh1")
                for th in range(2):
                    pp = psB.tile([3, 1024], F32, name="pp", tag="pp")
                    for tt2 in range(2):
                        tt = th * 2 + tt2
                        t0 = tt * 512
                        xn = xin.tile([128, 1024], F32, name="xn", tag="xn")
                        base = (bi * T + t0) * D
                        nc.sync.dma_start(
                            xn[:],
                            bass.AP(x_d, base,
                                    [[D, 128], [128 * D, 4], [1, D]]),
                        )
                        for dc in range(2):
                            xt_ps = psA.tile([128, 512], F32, name="xt",
                                             tag="xt")
                            for s in range(4):
                                nc.tensor.transpose(
                                    xt_ps[:, s * 128:(s + 1) * 128],
                                    xn[:, s * 256 + dc * 128:
                                       s * 256 + (dc + 1) * 128],
                                    ident[:],
                                )
                            xt_sb = xts.tile([128, 512], F32, name="xs",
                                             tag="xs")
                            nc.scalar.copy(xt_sb[:], xt_ps[:])
                            nc.tensor.matmul(pp[:, tt2 * 512:tt2 * 512 + 512],
                                             w_sb[:, dc * 3:(dc + 1) * 3],
                                             xt_sb[:],
                                             start=(dc == 0), stop=(dc == 1))
                    nc.scalar.copy(pp_sb[:, th * 1024:(th + 1) * 1024], pp[:])
                for j in range(3):
                    nc.sync.dma_start(heads[j][bi:bi + 1, :], pp_sb[j:j + 1, :])

            # ---------------- P2: finalize preds --------------------------
            mask = sb.tile([R, T], F32)
            nc.vector.tensor_scalar(mask[:], sal_sb[:], 0.0, None, op0=A.is_ge)
            window, offset = wlog, olog
            # sigmoid(x) = 1/(1+exp(-x)) with IEEE reciprocal for accuracy
            nbias = cst.tile([R, 1], F32)
            nc.vector.tensor_scalar(nbias[:], brep_sb[:, 0:1], -1.0, None,
                                    op0=A.mult)
            center = sb.tile([R, T], F32)
            nc.scalar.activation(center[:], clog[:], AF.Exp, bias=nbias[:],
                                 scale=-1.0)
            nc.vector.tensor_scalar(center[:], center[:], 1.0, None, op0=A.add)
            nc.vector.reciprocal(center[:], center[:])
            nc.vector.tensor_tensor(center[:], center[:], mask[:], op=A.mult)
            nc.vector.tensor_scalar(window[:], wlog[:], brep_sb[:, 1:2], None,
                                    op0=A.add)
            nc.vector.tensor_scalar(offset[:], olog[:], brep_sb[:, 2:3], None,
                                    op0=A.add)
            # ranking key: biased logit, masked -> -1e30 (sigmoid is monotone,
            # so NMS + topk order on logits == order on sigmoid outputs)
            keym = clog
            mneg = tmp.tile([R, T], F32, name="mneg", tag="D")
            nc.vector.tensor_scalar(mneg[:], mask[:], 1e30, -1e30, op0=A.mult,
                                    op1=A.add)
            nc.vector.tensor_scalar(keym[:], clog[:], brep_sb[:, 0:1], None,
                                    op0=A.add)
            nc.vector.tensor_tensor(keym[:], keym[:], mneg[:], op=A.add)

            # ---------------- P3: NMS -------------------------------------
            cpad = tmp.tile([R, T + 2], F32, name="cpad", tag="F")
            nc.gpsimd.memset(cpad[:], NEG)
            nc.scalar.copy(cpad[:, 1:T + 1], keym[:])
            hmax = tmp.tile([R, T], F32, name="hmax", tag="D")
            nc.vector.tensor_tensor(hmax[:], cpad[:, 0:T], cpad[:, 1:T + 1],
                                    op=A.max)
            nc.vector.tensor_tensor(hmax[:], hmax[:], cpad[:, 2:T + 2], op=A.max)
            nc.vector.tensor_tensor(hmax[:], hmax[:], keym[:], op=A.is_equal)
            neg3 = tmp.tile([R, T], F32, name="neg3", tag="E")
            nc.vector.tensor_scalar(neg3[:], hmax[:], 1e30, -1e30, op0=A.mult,
                                    op1=A.add)
            cpn = sb.tile([R, T], F32)
            nc.vector.tensor_tensor(cpn[:], keym[:], hmax[:], op=A.mult)
            nc.vector.tensor_tensor(cpn[:], cpn[:], neg3[:], op=A.add)

            # ---------------- P4: gaussian targets ------------------------
            ci160 = sb.tile([1, 160], F32)
            r2160 = sb.tile([1, 160], F32)
            niv160 = sb.tile([1, 160], F32)
            for g in range(2):
                bsc = sb.tile([80, 2], F32, name=f"bsc{g}", tag=f"bsc{g}")
                nc.sync.dma_start(
                    bsc[:],
                    bass.AP(bnd_d, g * 20, [[2 * M, R], [2, 10], [1, 2]]),
                )
                b0 = bsc[:, 0:1]
                b1 = bsc[:, 1:2]
                vld = sb.tile([80, 1], F32, name=f"vld{g}", tag=f"vld{g}")
                nc.vector.tensor_scalar(vld[:], b0, -1.0, None, op0=A.not_equal)
                b0c = sb.tile([80, 1], F32, name=f"b0c{g}", tag=f"b0c{g}")
                nc.vector.tensor_scalar(b0c[:], b0, 0.5, None, op0=A.mult)
                b1c = sb.tile([80, 1], F32, name=f"b1c{g}", tag=f"b1c{g}")
                nc.vector.tensor_scalar(b1c[:], b1, 2.0, 0.5, op0=A.subtract,
                                        op1=A.mult)
                ctr = sb.tile([80, 1], F32, name=f"ctr{g}", tag=f"ctr{g}")
                nc.vector.tensor_scalar(ctr[:], b0c[:], b1c[:], 0.5, op0=A.add,
                                        op1=A.mult)
                nc.vector.tensor_scalar(ctr[:], ctr[:], float(T) - 0.5, None,
                                        op0=A.min)
                win = sb.tile([80, 1], F32, name=f"win{g}", tag=f"win{g}")
                nc.vector.tensor_tensor(win[:], b1c[:], b0c[:], op=A.subtract)
                radf = sb.tile([80, 1], F32, name=f"radf{g}", tag=f"radf{g}")
                nc.vector.tensor_scalar(radf[:], win[:], 0.2, None, op0=A.mult)
                rad = _floor(nc, sb, radf[:], f"rad{g}")
                ci = _floor(nc, sb, ctr[:], f"ci{g}")
                sig = sb.tile([80, 1], F32, name=f"sig{g}", tag=f"sig{g}")
                nc.vector.tensor_scalar(sig[:], rad[:], 1.0, 0.2, op0=A.add,
                                        op1=A.mult)
                s22 = sb.tile([80, 1], F32, name=f"s22{g}", tag=f"s22{g}")
                nc.vector.scalar_tensor_tensor(s22[:], sig[:], 2.0, sig[:],
                                               op0=A.mult, op1=A.mult)
                ninv = sb.tile([80, 1], F32, name=f"ninv{g}", tag=f"ninv{g}")
                nc.vector.reciprocal(ninv[:], s22[:])
                nc.vector.tensor_scalar(ninv[:], ninv[:], -1.0, None, op0=A.mult)
                offv = sb.tile([80, 1], F32, name=f"offv{g}", tag=f"offv{g}")
                nc.vector.tensor_tensor(offv[:], ctr[:], ci[:], op=A.subtract)

                # r2v = valid ? r^2 : -1 (band test is exact in f32 ints)
                r2v = sb.tile([80, 1], F32, name=f"r2v{g}", tag=f"r2v{g}")
                nc.vector.tensor_tensor(r2v[:], rad[:], rad[:], op=A.mult)
                nc.vector.tensor_scalar(r2v[:], r2v[:], 1.0, None, op0=A.add)
                nc.vector.tensor_tensor(r2v[:], r2v[:], vld[:], op=A.mult)
                nc.vector.tensor_scalar(r2v[:], r2v[:], -1.0, None, op0=A.add)
                # collect coefficient rows [1, 160] in (b, m) b-major order
                for arr, row in ((ci, ci160), (r2v, r2160), (ninv, niv160)):
                    for b in range(R):
                        nc.sync.dma_start(
                            row[0:1, b * 20 + g * 10:b * 20 + g * 10 + 10],
                            arr[b * 10:(b + 1) * 10, :])
                inval = sb.tile([80, 1], F32, name=f"inval{g}", tag=f"inval{g}")
                nc.vector.tensor_scalar(inval[:], vld[:], -1e6, 1e6,
                                        op0=A.mult, op1=A.add)
                voff = sb.tile([80, 1], F32, name=f"voff{g}", tag=f"voff{g}")
                nc.vector.tensor_scalar(voff[:], ci[:], rb2048[:], inval[:],
                                        op0=A.add, op1=A.add)
                vofu = sb.tile([80, 1], U32, name=f"vofu{g}", tag=f"vofu{g}")
                nc.vector.tensor_copy(vofu[:], voff[:])

                # value scatters (last-write-wins; invalid offsets OOB-drop)
                ones = sb.tile([80, 1], F32, name=f"ones{g}", tag=f"ones{g}")
                nc.vector.memset(ones[:], 1.0)
                for dst, src in ((wtg_d, win), (otg_d, offv), (wgt_d, ones)):
                    nc.gpsimd.indirect_dma_start(
                        out=dst[:].rearrange("(a k) -> a k", k=1),
                        out_offset=IndirectOffsetOnAxis(ap=vofu[:], axis=0),
                        in_=src[:],
                        in_offset=None,
                        bounds_check=R * T - 1,
                        oob_is_err=False,
                        compute_op=A.bypass,
                    )

            # dense gaussian field: partitions = t % 128, free = (b, tc, m)
            F32R = mybir.dt.float32r
            ct_tp = sb.tile([128, 128], F32)
            tci = sb.tile([1, 1280], I32, name="tci", tag="gb")
            nc.gpsimd.iota(tci[:], [[0, R], [-128, 8], [0, 20]],
                           base=0, channel_multiplier=0)
            tcf = cst.tile([1, 1280], F32)
            nc.vector.tensor_copy(tcf[:], tci[:])
            for h in range(2):
                # coefficient rows for this half, tc-replicated
                cip = sb.tile([1, 1280], F32, name=f"cip{h}", tag="cip")
                r2r = sb.tile([1, 1280], F32, name=f"r2r{h}", tag="r2r")
                nivr = sb.tile([1, 1280], F32, name=f"nivr{h}", tag="nivr")
                for tc in range(8):
                    for row, dstv in ((ci160, cip), (r2160, r2r),
                                      (niv160, nivr)):
                        nc.sync.dma_start(
                            dstv[:].rearrange("k (b c m) -> k b c m",
                                              b=R, m=M)[:, :, tc:tc + 1, :],
                            row[:].rearrange("k (b m) -> k b m", b=R))
                # cip -= 128 * tc_global  (tcf holds -128*tc_local)
                nc.vector.scalar_tensor_tensor(cip[:], tcf[:],
                                               float(-1024 * h), cip[:],
                                               op0=A.add, op1=A.add)

                NS = (512, 512, 256)
                dh = sb.tile([128, 1280], F32, name=f"dh{h}", tag="gd")
                bandh = sb.tile([128, 1280], F32, name=f"bh{h}", tag="gb")
                cig = psC.tile([128, 1280], F32, name=f"cig{h}", tag="gau")
                o = 0
                for n in NS:
                    nc.tensor.matmul(cig[:, o:o + n], ones1[:],
                                     cip[:, o:o + n], start=True, stop=True)
                    o += n
                nc.vector.tensor_scalar(dh[:], cig[:], tpcol[:], None,
                                        op0=A.subtract)
                r2g = psC.tile([128, 1280], F32, name=f"r2g{h}", tag="gau")
                o = 0
                for n in NS:
                    nc.tensor.matmul(r2g[:, o:o + n], ones1[:],
                                     r2r[:, o:o + n], start=True, stop=True)
                    o += n
                nc.vector.tensor_tensor(dh[:], dh[:], dh[:], op=A.mult)
                nc.vector.tensor_tensor(bandh[:], dh[:], r2g[:], op=A.is_le)
                nig = psC.tile([128, 1280], F32, name=f"nig{h}", tag="gau")
                o = 0
                for n in NS:
                    nc.tensor.matmul(nig[:, o:o + n], ones1[:],
                                     nivr[:, o:o + n], start=True, stop=True)
                    o += n
                nc.vector.tensor_tensor(dh[:], dh[:], nig[:], op=A.mult)
                nc.scalar.activation(dh[:], dh[:], AF.Exp)
                nc.vector.tensor_tensor(dh[:], dh[:], bandh[:], op=A.mult)
                # max over m: free view [128, 8b, 8tc, 20m] -> [128, 8b, 8tc]
                nc.vector.tensor_reduce(
                    bass.AP(ct_tp.tensor, ct_tp[:].offset + h * 8,
                            [[128, 128], [16, R], [1, 8]]),
                    dh[:].rearrange("p (b c m) -> p b c m", b=R, m=M),
                    axis=AX.X, op=A.max)
            # transpose [tp, (b, tc)] -> [(b, tc), tp] and regroup rows
            trp = psC.tile([128, 128], F32, name="trp", tag="gau")
            nc.tensor.transpose(trp[:], ct_tp[:], ident[:])
            ctT = sb.tile([128, 128], F32)
            nc.scalar.copy(ctT[:], trp[:])
            ct = sb.tile([R, T], F32, name="ct", tag="big2")
            for bi in range(R):
                nc.sync.dma_start(ct[bi:bi + 1, :],
                                  ctT[bi * 16:(bi + 1) * 16, :])
            wtg = sb.tile([R, T], F32, name="wtg", tag="sh1")
            nc.sync.dma_start(wtg[:], wtg_d[:].rearrange("(r t) -> r t", t=T))
            otg = sb.tile([R, T], F32, name="otg", tag="sh2")
            nc.sync.dma_start(otg[:], otg_d[:].rearrange("(r t) -> r t", t=T))
            wgt = sb.tile([R, T], F32, name="wgt", tag="sh3")
            nc.sync.dma_start(wgt[:], wgt_d[:].rearrange("(r t) -> r t", t=T))

            # ---------------- P5: loss partials ---------------------------
            part = sb.tile([R, 4], F32)
            a1 = tmp.tile([R, T], F32, name="a1", tag="Aa")
            nc.scalar.activation(a1[:], center[:], AF.Ln, bias=epsb[:], scale=1.0)
            om = tmp.tile([R, T], F32, name="om", tag="Bb")
            nc.scalar.activation(om[:], center[:], AF.Copy, bias=1.0, scale=-1.0)
            som = tmp.tile([R, T], F32, name="som", tag="Cc")
            nc.scalar.activation(som[:], om[:], AF.Square)
            tpos = tmp.tile([R, T], F32, name="tpos", tag="D")
            nc.vector.tensor_tensor(tpos[:], a1[:], som[:], op=A.mult)
            nc.vector.tensor_tensor(tpos[:], tpos[:], wgt[:], op=A.mult)

            b1l = tmp.tile([R, T], F32, name="b1l", tag="Aa")
            nc.scalar.activation(b1l[:], center[:], AF.Ln, bias=lneps[:],
                                 scale=-1.0)
            b2 = tmp.tile([R, T], F32, name="b2", tag="Bb")
            nc.scalar.activation(b2[:], center[:], AF.Square)
            ctm = tmp.tile([R, T], F32, name="ctm", tag="Cc")
            nc.scalar.activation(ctm[:], ct[:], AF.Copy, bias=1.0, scale=-1.0)
            nc.scalar.activation(ctm[:], ctm[:], AF.Square)
            nc.scalar.activation(ctm[:], ctm[:], AF.Square)
            tneg = tmp.tile([R, T], F32, name="tneg", tag="E")
            nc.vector.tensor_tensor(tneg[:], b1l[:], b2[:], op=A.mult)
            nc.vector.tensor_tensor(tneg[:], tneg[:], ctm[:], op=A.mult)
            nc.vector.tensor_tensor(tneg[:], tneg[:], tpos[:], op=A.add)
            nc.vector.scalar_tensor_tensor(tneg[:], tneg[:], 1.0, mask[:],
                                           op0=A.mult, op1=A.mult,
                                           accum_out=part[:, 0:1])

            for col, (pred, tgt) in ((1, (window, wtg)), (2, (offset, otg))):
                df = tmp.tile([R, T], F32, name=f"df{col}", tag="E")
                nc.vector.tensor_tensor(df[:], pred[:], tgt[:], op=A.subtract)
                nc.scalar.activation(df[:], df[:], AF.Abs)
                nc.vector.scalar_tensor_tensor(df[:], df[:], 1.0, wgt[:],
                                               op0=A.mult, op1=A.mult,
                                               accum_out=part[:, col:col + 1])

            # valid count per row
            bnd8 = sb.tile([R, 2 * M], F32)
            nc.sync.dma_start(bnd8[:], bnd_d[:].rearrange("r m c -> r (m c)"))
            vld8 = sb.tile([R, M], F32)
            nc.vector.tensor_scalar(
                vld8[:].rearrange("p (m k) -> p m k", k=1),
                bnd8[:].rearrange("p (m c) -> p m c", c=2)[:, :, 0:1],
                -1.0, None, op0=A.not_equal)
            nc.vector.tensor_reduce(part[:, 3:4], vld8[:], axis=AX.X, op=A.add)
            nc.sync.dma_start(opart_d[:], part[:])

            # ---------------- P6: topk over pair-reduced array ------------
            pm = sb.tile([R, T // 2], F32, name="pm", tag="gd")
            cpn_pr = cpn[:].rearrange("p (i s) -> p i s", s=2)
            nc.vector.tensor_tensor(
                pm[:].rearrange("p (i k) -> p i k", k=1),
                cpn_pr[:, :, 0:1], cpn_pr[:, :, 1:2], op=A.max)
            work = sb.tile([R, T // 2], F32, name="work", tag="gb")
            nc.vector.tensor_copy(work[:], pm[:])
            vals = sb.tile([R, K104], F32)
            idx = sb.tile([R, K104], U32)
            for r in range(NR):
                m8 = vals[:, r * 8:(r + 1) * 8]
                nc.vector.max(out=m8, in_=work[:])
                nc.vector.max_index(idx[:, r * 8:(r + 1) * 8], m8, work[:])
                nc.vector.match_replace(out=work[:], in_to_replace=m8,
                                        in_values=work[:], imm_value=NEG)

            # ---------------- P7: decode ----------------------------------
            wplus = tmp.tile([R, T], F32, name="wplus", tag="Aa")
            nc.vector.tensor_scalar(wplus[:], window[:], 0.0, None, op0=A.max)
            oplus = tmp.tile([R, T], F32, name="oplus", tag="Bb")
            nc.vector.tensor_scalar(oplus[:], offset[:], 0.0, None, op0=A.max)
            ctr2 = tmp.tile([R, T], F32, name="ctr2", tag="F")
            nc.gpsimd.iota(ctr2[:], [[2, T]], base=0, channel_multiplier=0,
                           allow_small_or_imprecise_dtypes=True)
            nc.vector.scalar_tensor_tensor(ctr2[:], oplus[:], 2.0, ctr2[:],
                                           op0=A.mult, op1=A.add)
            pay = sb.tile([R, T * 3], F32, name="pay", tag="big")

            def pay_view(c0, n):
                return pay[:].rearrange("p (i k) -> p i k", k=6)[:, :, c0:c0 + n]

            def half_view(t_, c0, n):
                return t_[:].rearrange("p (i s) -> p i s", s=2)[:, :, c0:c0 + n]

            # cp pairs -> payload cols 0:2
            nc.scalar.copy(pay_view(0, 2), half_view(cpn, 0, 2))
            # lo = clamp(2t + 2o' - w', 0, 4094) -> cols 2:4
            lraw = tmp.tile([R, T], F32, name="lraw", tag="Cc")
            nc.vector.scalar_tensor_tensor(lraw[:], wplus[:], -1.0, ctr2[:],
                                           op0=A.mult, op1=A.add)
            nc.vector.tensor_scalar(pay_view(2, 2), half_view(lraw, 0, 2),
                                    0.0, 4094.0, op0=A.max, op1=A.min)
            # hi = clamp(2t + 2o' + w', 0, 4094) + 2 -> cols 4:6
            hraw = tmp.tile([R, T], F32, name="hraw", tag="Cc")
            nc.vector.tensor_tensor(hraw[:], ctr2[:], wplus[:], op=A.add)
            nc.vector.tensor_scalar(hraw[:], hraw[:], 0.0, 4094.0,
                                    op0=A.max, op1=A.min)
            nc.scalar.activation(pay_view(4, 2), half_view(hraw, 0, 2),
                                 AF.Identity, bias=twob[:], scale=1.0)
            nc.sync.dma_start(pay_d[:].rearrange("(r t) -> r t", t=T * 3),
                              pay[:])

            idxf = sb.tile([R, K104], F32)
            nc.vector.tensor_copy(idxf[:], idx[:])
            nc.vector.tensor_scalar(idxf[:], idxf[:], rb1024[:], None, op0=A.add)
            goff = sb.tile([R, K104], U32)
            nc.vector.tensor_copy(goff[:], idxf[:])
            # HW vector-indirect DMA = ONE offset per partition: per-b gather
            # with ranks on partitions
            g8 = sb.tile([K104, 64], F32, name="g8", tag="sh3")
            payv = pay_d[:].rearrange("(a k) -> a k", k=6)
            for b in range(R):
                offb = sb.tile([K104, 1], U32, name=f"offb{b}", tag="offb")
                nc.sync.dma_start(offb[:], goff[b:b + 1, :])
                nc.gpsimd.indirect_dma_start(
                    out=g8[:, b * 8:b * 8 + 6],
                    out_offset=None,
                    in_=payv,
                    in_offset=IndirectOffsetOnAxis(ap=offb[:], axis=0),
                )

            def g8v(c0):
                return g8[:].rearrange("p (b k) -> p b k", k=8)[:, :, c0:c0 + 1]

            sel = sb.tile([K104, R], F32)
            selv = sel[:].rearrange("p (b k) -> p b k", k=1)
            nc.vector.tensor_tensor(selv, g8v(1), g8v(0), op=A.is_gt)
            out3 = sb.tile([R, TOPK * 3], F32, name="out3", tag="big2")

            def o3v(c0):
                return out3[:].rearrange("p (i k) -> p i k", k=3)[:, :, c0:c0 + 1]

            lh = []
            for nm, base in (("lo_a", 2), ("hi_a", 4)):
                dt_ = sb.tile([K104, R], F32, name=nm, tag=nm)
                dtv = dt_[:].rearrange("p (b k) -> p b k", k=1)
                nc.vector.tensor_tensor(dtv, g8v(base + 1), g8v(base),
                                        op=A.subtract)
                nc.vector.tensor_tensor(dtv, dtv, selv, op=A.mult)
                nc.vector.tensor_tensor(dtv, dtv, g8v(base), op=A.add)
                lh.append(dt_)
            # assemble: rank r on partition r, batch b on column b
            for c0, dt_ in ((0, lh[0]), (1, lh[1])):
                for b in range(R):
                    a = out3[b:b + 1, c0:c0 + 1]
                    dst = bass.AP(a.tensor, a.offset,
                                  [list(a.ap[0]), [3, TOPK]])
                    nc.sync.dma_start(dst, dt_[0:TOPK, b:b + 1])
            scs = sb.tile([R, K104], F32)
            nc.scalar.activation(scs[:], vals[:], AF.Exp, scale=-1.0)
            nc.vector.tensor_scalar(scs[:], scs[:], 1.0, None, op0=A.add)
            nc.vector.reciprocal(scs[:], scs[:])
            nc.scalar.copy(
                o3v(2), scs[:, 0:TOPK].rearrange("p (i k) -> p i k", k=1))
            nc.sync.dma_start(obnd_d[:], out3[:])

    nc.compile()
    return nc


_NC_CACHE = None


def _run_spmd(inputs, trace=False):
    global _NC_CACHE
    if _NC_CACHE is None:
        _NC_CACHE = build_nc()
    nc = _NC_CACHE

    x = np.ascontiguousarray(inputs["x"], dtype=np.float32)
    saliency = np.ascontiguousarray(inputs["saliency"], dtype=np.float32)
    boundary = np.ascontiguousarray(inputs["boundary"], dtype=np.float32)
    wpk = np.ascontiguousarray(
        np.concatenate([inputs["w_center"], inputs["w_window"],
                        inputs["w_offset"]], axis=1), dtype=np.float32)
    brep = np.broadcast_to(
        np.array([inputs["b_center"][0], inputs["b_window"][0],
                  inputs["b_offset"][0]], dtype=np.float32), (R, 3)).copy()

    in_maps = []
    for c in range(NCORES):
        s = slice(c * R, (c + 1) * R)
        in_maps.append({
            "x": x[s],
            "sal": saliency[s],
            "bnd": boundary[s],
            "wpk": wpk,
            "brep": brep,
        })
    return run_bass_kernel_spmd(nc, in_maps, core_ids=list(range(NCORES)),
                                trace=trace)


def kernel(x, w_center, b_center, w_window, b_window, w_offset, b_offset,
           saliency, boundary):
    res = _run_spmd(dict(x=x, w_center=w_center, b_center=b_center,
                         w_window=w_window, b_window=b_window,
                         w_offset=w_offset, b_offset=b_offset,
                         saliency=saliency, boundary=boundary))
    obnd = np.concatenate([r["obnd"] for r in res.results], axis=0)
    parts = np.stack([r["opart"] for r in res.results])  # [8, R, 4]
    tot = parts.sum(axis=(0, 1))
    avg = tot[3]
    center_loss = np.float32(-tot[0] / avg)
    window_loss = np.float32(0.1 * tot[1] / avg)
    offset_loss = np.float32(1.0 * tot[2] / avg)
    return (obnd, center_loss, window_loss, offset_loss)
